# revision 10
# baseline (speedup 1.0000x reference)
"""Multi-head cross-attention Trainium2 kernel (8-core SPMD), v3.

Sharding: 2 batch groups x 4 cores. Core c handles batch b = c // 4 and
heads [4*(c%4), 4*(c%4)+4). Each core computes its 4 heads' attention
output and a partial output projection (row-sharded Wp); the host sums
the 4 partials per batch (the all-reduce step of tensor parallelism).

All matmul operands are fp16 (fp32 matmul is 4x slower), accumulation is
fp32 in PSUM. Output y is written fp16 (error budget 2e-2 >> fp16 eps).

The kernel is PE-issue-bound (~215ns per 512-col matmul slot, LDWEIGHTS
hidden) with ACT exp (~1.11us per s-tile) a close second. v3 removes all
non-essential PE slots and overlaps the head/tail:
 - per-tq/per-sc chunked input DMAs; Q/KV priming starts ~4us in.
 - PE warm-up matmuls during the DMA window (HAM K=8/8 by first real MM).
 - V^T -> V_aug via PE transpose + DVE copies (DMA-transpose XBAR costs
   ~1.2us of serial DGE-queue time per 64-col transpose -- too slow).
 - softmax denominator broadcast via DRAM-bounce DMA on the gpsimd queue
   (frees 16 PE broadcast matmuls and all tq-boundary PSUM pressure).
 - exp ACT table preloaded at t=0; fp16 output with per-t-tile DMAs.
 - deferred Q/KV/outproj work split into ~1-matmul items popped into the
   exp slack, with deadline-forced emission (+2 iteration margin).

Structure per core ("mh" = head pair, 2 per core):
  Q^T[d,t] = Wq4.T @ xT (+bias)  per (mh, tq) chunk
  K^T/V^T chunks; V_aug[s, 64|1(ones)] via DMA-transpose
  attention (ACT-bound): att[s-tile] = [h0 | h1] joint PSUM [128,1024],
    E = exp(att/32) one ACT instr, fp16; U_aug[65,tq] += V_aug.T @ E
    (softmax denom rides in row 64).
  normalize: U evac, DRAM-bounce denom broadcast, recip, mul.
  outproj: y_ps[n] += UN[mh].T @ WpT; fp16 evac; per-t-tile DMA.

PSUM: "att" 2x2 banks + "uh" 2x1 + "kv" 2x1 = 8 banks.
"""

import os
import numpy as np
from collections import deque
from contextlib import ExitStack

import concourse.bass as bass
import concourse.bacc as bacc
import concourse.tile as tile
from concourse import mybir
from concourse.bass_utils import run_bass_kernel_spmd
from concourse.masks import make_identity

F32 = mybir.dt.float32
F16 = mybir.dt.float16
AF = mybir.ActivationFunctionType

B, T, S, C = 2, 2048, 2048, 1024
H, HD = 16, 64
NCORES = 8
HPC = 4            # heads per core
MHN = 2            # head-pairs per core
KC = C // 128      # 8 contraction tiles
STILES = S // 128  # 16
TTILES = T // 16   # 128
TQN = 4            # t-quarters of 512
SCN = 4            # s-chunks of 512
SCALE = 1.0 / np.sqrt(C)
WARMUP_MMS = 12    # PE warm-up matmuls during the input-DMA window

LAST_RESULTS = None
_NC_CACHE = None


def _build_nc():
    nc = bacc.Bacc()

    xT4 = nc.declare_dram_parameter("xT4", [128, TQN, KC, 512], F16, isOutput=False)
    eT4 = nc.declare_dram_parameter("eT4", [128, SCN, KC, 512], F16, isOutput=False)
    Wq4 = nc.declare_dram_parameter("Wq4", [128, KC, 256], F16, isOutput=False)
    Wk4 = nc.declare_dram_parameter("Wk4", [128, KC, 256], F16, isOutput=False)
    Wv4 = nc.declare_dram_parameter("Wv4", [128, KC, 256], F16, isOutput=False)
    b6 = nc.declare_dram_parameter("b6", [128, 6], F32, isOutput=False)
    WpT4 = nc.declare_dram_parameter("WpT4", [128, 2, C], F16, isOutput=False)
    y = nc.declare_dram_parameter("y", [T, C], F16, isOutput=True)
    # denominator DRAM-bounce scratch, one slot per (mh, tq)
    dscr = nc.declare_dram_parameter("dscr", [MHN * TQN, 1024], F32,
                                     isOutput=True)

    with tile.TileContext(nc) as tc, ExitStack() as ctx:
        consts = ctx.enter_context(tc.tile_pool(name="consts", bufs=1))
        wpool = ctx.enter_context(tc.tile_pool(name="wts", bufs=1))
        qkvp = ctx.enter_context(tc.tile_pool(name="qkvt", bufs=2))
        vtsp = ctx.enter_context(tc.tile_pool(name="vts", bufs=2))
        vap = ctx.enter_context(tc.tile_pool(name="vaug", bufs=2))
        epool = ctx.enter_context(tc.tile_pool(name="esb", bufs=4))
        unp = ctx.enter_context(tc.tile_pool(name="unorm", bufs=2))
        usbp = ctx.enter_context(tc.tile_pool(name="usb", bufs=3))
        dnp = ctx.enter_context(tc.tile_pool(name="denom", bufs=2))
        ysbp = ctx.enter_context(tc.tile_pool(name="ysb", bufs=3))
        psp = ctx.enter_context(tc.tile_pool(name="ps", bufs=2, space="PSUM"))

        # ---- t=0: ACT exp table preload ----
        dummy = consts.tile([1, 8], F32, tag="dummy", name="dummy")
        nc.vector.memset(dummy, 0.0)
        nc.scalar.activation(dummy, dummy, AF.Exp, scale=1.0)

        ident = consts.tile([128, 128], F16)
        make_identity(nc, ident)

        b6sb = consts.tile([128, 6], F32, tag="b6", name="b6sb")
        nc.sync.dma_start(out=b6sb, in_=b6[:, :])
        bsb = {"q": b6sb[:, 0:2], "k": b6sb[:, 2:4], "v": b6sb[:, 4:6]}

        # ---- input DMAs, ordered by first consumer ----
        wsb = {}
        wsb["q"] = wpool.tile([128, KC, 256], F16, tag="wq", name="wqsb")
        nc.sync.dma_start(out=wsb["q"], in_=Wq4[:, :, :])
        # chunk-major: each chunk DMA dst is one contiguous 8KB run per
        # partition (strided dsts cost ~3.7us of serial DGE queue time)
        xt_sb = wpool.tile([128, TQN, KC, 512], F16, tag="xt")
        et_sb = wpool.tile([128, SCN, KC, 512], F16, tag="et")
        nc.sync.dma_start(out=xt_sb[:, 0], in_=xT4[:, 0])
        wsb["k"] = wpool.tile([128, KC, 256], F16, tag="wk", name="wksb")
        nc.sync.dma_start(out=wsb["k"], in_=Wk4[:, :, :])
        nc.sync.dma_start(out=et_sb[:, 0], in_=eT4[:, 0])
        wsb["v"] = wpool.tile([128, KC, 256], F16, tag="wv", name="wvsb")
        nc.sync.dma_start(out=wsb["v"], in_=Wv4[:, :, :])
        for j in range(1, 4):
            nc.sync.dma_start(out=xt_sb[:, j], in_=xT4[:, j])
            nc.sync.dma_start(out=et_sb[:, j], in_=eT4[:, j])
        wpt = wpool.tile([128, 2, C], F16, tag="wpt")
        nc.sync.dma_start(out=wpt, in_=WpT4[:, :, :])

        # ---- PE warm-up: keep HAM at K=8/8 through the DMA window ----
        wz = consts.tile([128, 512], F16, tag="wz", name="wz")
        nc.vector.memset(wz, 0.0)
        warm_ps = psp.tile([128, 512], F32, tag="kv", name="warm_ps")
        for _ in range(WARMUP_MMS):
            nc.tensor.matmul(warm_ps, wz[:, 0:128], wz,
                             start=True, stop=True)

        # ---- persistent SBUF tensors ----
        QT = [qkvp.tile([128, T], F16, tag="qt", name=f"QT{i}")
              for i in range(MHN)]
        KT = [qkvp.tile([128, S], F16, tag="kt", name=f"KT{i}")
              for i in range(MHN)]
        # 256-wide rows: DMA-transpose dests must sit at 128B multiples
        # (the XBAR shifts writes +16B otherwise). h0 V at 0:64 (+ones
        # col 64), h1 V at 128:192 (+ones col 192).
        VA = [vap.tile([128, STILES, 256], F16, tag="va", name=f"va{i}")
              for i in range(MHN)]
        for i in range(MHN):
            nc.gpsimd.memset(VA[i][:, :, 64:65], 1.0)
            nc.gpsimd.memset(VA[i][:, :, 192:193], 1.0)
        UN = [unp.tile([128, T], F16, tag="un", name=f"UN{i}")
              for i in range(MHN)]

        def q_pass(mh, tq):
            """Generator of work items: Q^T for one (mh, tq)."""
            state = {}

            def mk_alloc():
                state["ps"] = psp.tile([128, 512], F32, tag="kv",
                                       name=f"qtps{mh}_{tq}")

            yield mk_alloc
            for k in range(KC):
                def mk_k(k=k):
                    nc.tensor.matmul(state["ps"],
                                     wsb["q"][:, k, mh * 128:(mh + 1) * 128],
                                     xt_sb[:, tq, k],
                                     start=(k == 0), stop=(k == KC - 1))

                yield mk_k

            def mk_evac():
                nc.vector.tensor_scalar_add(
                    out=QT[mh][:, tq * 512:(tq + 1) * 512],
                    in0=state["ps"], scalar1=bsb["q"][:, mh:mh + 1])

            yield mk_evac

        def kv_work(mh, sc):
            """Generator of work items: K^T/V^T chunk + V DMA-transposes."""
            csl = slice(sc * 512, (sc + 1) * 512)
            state = {}

            def mk_alloc():
                state["kt_ps"] = psp.tile([128, 512], F32, tag="kv",
                                          name=f"ktps{mh}_{sc}")
                state["vt_ps"] = psp.tile([128, 512], F32, tag="kv",
                                          name=f"vtps{mh}_{sc}")

            yield mk_alloc

            for nm in ("k", "v"):
                for k in range(KC):
                    def mk_k(nm=nm, k=k):
                        ps = state["kt_ps"] if nm == "k" else state["vt_ps"]
                        nc.tensor.matmul(ps,
                                         wsb[nm][:, k, mh * 128:(mh + 1) * 128],
                                         et_sb[:, sc, k],
                                         start=(k == 0), stop=(k == KC - 1))

                    yield mk_k

            def mk_evac():
                nc.vector.tensor_scalar_add(out=KT[mh][:, csl],
                                            in0=state["kt_ps"],
                                            scalar1=bsb["k"][:, mh:mh + 1])
                vts = vtsp.tile([128, 512], F16, tag="vts",
                                name=f"vts{mh}_{sc}")
                nc.vector.tensor_scalar_add(out=vts, in0=state["vt_ps"],
                                            scalar1=bsb["v"][:, mh:mh + 1])
                state["vts"] = vts

            yield mk_evac

            def mk_tr(jj):
                for j in (jj, jj + 1):
                    s = sc * 4 + j
                    blk = slice(j * 128, (j + 1) * 128)
                    if mh == 1:
                        # long lead time: idle sync-queue DMA-transpose
                        nc.sync.dma_start(out=VA[mh][:, s, 0:64],
                                          in_=state["vts"][0:64, blk],
                                          transpose=True)
                        nc.sync.dma_start(out=VA[mh][:, s, 128:192],
                                          in_=state["vts"][64:128, blk],
                                          transpose=True)
                    else:
                        vtr = psp.tile([128, 128], F16, tag="kv",
                                       name=f"vtr{mh}_{s}")
                        nc.tensor.transpose(
                            vtr, state["vts"][:, blk], ident)
                        nc.vector.tensor_copy(VA[mh][:, s, 0:64],
                                              vtr[:, 0:64])
                        nc.vector.tensor_copy(VA[mh][:, s, 128:192],
                                              vtr[:, 64:128])

            yield (lambda: mk_tr(0))
            yield (lambda: mk_tr(2))

        y_r = y.rearrange("(tt p) o -> tt p o", p=128)

        def outproj_work(tq):
            """Deferred: partial out-projection for one t-quarter,
            per t-tile: 2 matmul items + 1 evac item + 1 dma item."""
            for j in range(4):
                t = tq * 4 + j
                state = {}

                def mk_mm(n, t=t, state=state):
                    if n == 0:
                        state["ps"] = [
                            psp.tile([128, 512], F32, tag="kv",
                                     name=f"yps{t}_{i}") for i in range(2)]
                    for mh in range(MHN):
                        nc.tensor.matmul(
                            state["ps"][n], UN[mh][:, t * 128:(t + 1) * 128],
                            wpt[:, mh, n * 512:(n + 1) * 512],
                            start=(mh == 0), stop=(mh == MHN - 1))

                def mk_evac(t=t, state=state):
                    ysb = ysbp.tile([128, 1024], F16, tag="ysb",
                                    name=f"ysb{t}")
                    nc.vector.tensor_copy(ysb[:, 0:512], state["ps"][0])
                    nc.vector.tensor_copy(ysb[:, 512:1024], state["ps"][1])
                    state["ysb"] = ysb

                def mk_dma(t=t, state=state):
                    nc.sync.dma_start(out=y_r[t], in_=state["ysb"])

                yield (lambda f=mk_mm: f(0))
                yield (lambda f=mk_mm: f(1))
                yield mk_evac
                yield mk_dma

        def normalize_work(mh, tq, uh0, uh1):
            """Deferred: evac U, DRAM-bounce denom broadcast, recip, mul."""
            qsl = slice(tq * 512, (tq + 1) * 512)
            slot = mh * TQN + tq
            state = {}

            def mk_evac0():
                usb = usbp.tile([65, 1024], F32, tag="usb",
                                name=f"usb{mh}_{tq}")
                nc.vector.tensor_copy(usb[:, 0:512], uh0)
                state["usb"] = usb

            def mk_evac1():
                nc.vector.tensor_copy(state["usb"][:, 512:1024], uh1)

            def mk_bounce():
                # denom row 64 -> DRAM -> broadcast-read to 64 partitions
                nc.gpsimd.dma_start(out=dscr[slot:slot + 1, :],
                                    in_=state["usb"][64:65, :])
                bc = dnp.tile([64, 1024], F32, tag="bc", bufs=2,
                              name=f"bc{mh}_{tq}")
                nc.gpsimd.dma_start(
                    out=bc, in_=dscr[slot:slot + 1, :].to_broadcast([64, 1024]))
                state["bc"] = bc

            def mk_recip():
                rbc = dnp.tile([64, 1024], F32, tag="rbc", bufs=2,
                               name=f"rbc{mh}_{tq}")
                nc.vector.reciprocal_approx_fast(rbc, state["bc"])
                state["rbc"] = rbc

            def mk_div0():
                nc.vector.tensor_mul(UN[mh][0:64, qsl],
                                     state["usb"][0:64, 0:512],
                                     state["rbc"][:, 0:512])

            def mk_div1():
                tmp1 = dnp.tile([64, 512], F16, tag="tmp1", bufs=2,
                                name=f"tmp1_{mh}_{tq}")
                nc.vector.tensor_mul(tmp1, state["usb"][0:64, 512:1024],
                                     state["rbc"][:, 512:1024])
                nc.gpsimd.dma_start(out=UN[mh][64:128, qsl], in_=tmp1)

            yield mk_evac0
            yield mk_evac1
            yield mk_bounce
            yield mk_recip
            yield mk_div0
            yield mk_div1

        emitted = set()

        def pop1(bulk):
            k, fn = bulk.popleft()
            fn()
            if not bulk or bulk[0][0] != k:
                emitted.add(k)

        def need(bulk, key):
            """Force-emit bulk items until producer group `key` is done."""
            while key not in emitted and bulk:
                pop1(bulk)

        def attention(mh, fast, bulk, bulk_budget):
            """ACT-bound attention for head pair mh; pops deferred work
            into the PE slack: 1 fast item + up to bulk_budget bulk."""
            for tq in range(TQN):
                need(bulk, ("q", mh, tq))
                qsl = slice(tq * 512, (tq + 1) * 512)
                uh0 = psp.tile([65, 512], F32, tag="uh", name=f"uh0_{mh}_{tq}")
                uh1 = psp.tile([65, 512], F32, tag="uh", name=f"uh1_{mh}_{tq}")
                prev_av = None
                for s in range(STILES):
                    # +2 iteration margin for the VA DMA-transpose latency
                    need(bulk, ("kv", mh, min((s + 2) // 4, SCN - 1)))
                    att = psp.tile([128, 1024], F32, tag="att",
                                   name=f"att_{mh}_{tq}_{s}")
                    ssl = slice(s * 128, (s + 1) * 128)
                    nc.tensor.matmul(att[:, 0:512], KT[mh][0:64, ssl],
                                     QT[mh][0:64, qsl], start=True, stop=True)
                    nc.tensor.matmul(att[:, 512:1024], KT[mh][64:128, ssl],
                                     QT[mh][64:128, qsl], start=True, stop=True)
                    ej = epool.tile([128, 1024], F16, tag="e",
                                    name=f"e_{mh}_{tq}_{s}")
                    nc.scalar.activation(ej, att, AF.Exp, scale=float(SCALE))
                    if prev_av is not None:
                        prev_av()
                    popped = 0
                    while fast and popped < 2:
                        fast.popleft()()
                        popped += 1
                    while bulk and popped < bulk_budget:
                        pop1(bulk)
                        popped += 1

                    def mk_av(s=s, ej=ej, uh0=uh0, uh1=uh1):
                        nc.tensor.matmul(uh0, VA[mh][:, s, 0:65], ej[:, 0:512],
                                         start=(s == 0), stop=(s == STILES - 1))
                        nc.tensor.matmul(uh1, VA[mh][:, s, 128:193],
                                         ej[:, 512:1024],
                                         start=(s == 0), stop=(s == STILES - 1))
                    prev_av = mk_av
                prev_av()
                fast.extend(normalize_work(mh, tq, uh0, uh1))
                if mh == MHN - 1:
                    fast.extend(outproj_work(tq))
            return fast, bulk

        # ---- priming: first chunks of both pairs while DMAs stream ----
        for w in q_pass(0, 0):
            w()
        for w in kv_work(0, 0):
            w()
        emitted.update({("q", 0, 0), ("kv", 0, 0)})

        # ---- deferred bulk, ordered by consumer deadline ----
        bulk = deque()
        for mh, kind, idx in (
            (1, "q", 0),
            (1, "kv", 0),
            (0, "kv", 1), (0, "kv", 2), (0, "kv", 3),
            (0, "q", 1),
            (1, "kv", 1),
            (0, "q", 2),
            (1, "kv", 2),
            (0, "q", 3),
            (1, "kv", 3),
            (1, "q", 1), (1, "q", 2), (1, "q", 3),
        ):
            key = (kind, mh, idx)
            gen = q_pass(mh, idx) if kind == "q" else kv_work(mh, idx)
            bulk.extend((key, w) for w in gen)

        fast = deque()
        fast, bulk = attention(0, fast, bulk, bulk_budget=2)
        fast, bulk = attention(1, fast, bulk, bulk_budget=2)
        while fast:
            fast.popleft()()
        while bulk:
            pop1(bulk)

    nc.compile()
    return nc


def _get_nc():
    global _NC_CACHE
    if _NC_CACHE is None:
        _NC_CACHE = _build_nc()
    return _NC_CACHE


def make_in_maps(e, x, Wq, bq, Wk, bk, Wv, bv, Wp):
    e = np.asarray(e, dtype=np.float32)
    x = np.asarray(x, dtype=np.float32)
    Wq, bq = np.asarray(Wq, np.float32), np.asarray(bq, np.float32)
    Wk, bk = np.asarray(Wk, np.float32), np.asarray(bk, np.float32)
    Wv, bv = np.asarray(Wv, np.float32), np.asarray(bv, np.float32)
    Wp = np.asarray(Wp, np.float32)

    def swiz(a2d):  # [C, N] -> [128, KC, N] partition-major
        Cd, N = a2d.shape
        return np.ascontiguousarray(
            a2d.reshape(KC, 128, N).transpose(1, 0, 2))

    def swiz4(a2d):  # [C, N] -> [128, 4, KC, 512] chunked partition-major
        return np.ascontiguousarray(
            a2d.reshape(KC, 128, 4, 512).transpose(1, 2, 0, 3))

    xTs = [swiz4(x[b].T.astype(np.float16)) for b in range(B)]
    eTs = [swiz4(e[b].T.astype(np.float16)) for b in range(B)]
    in_maps = []
    for c in range(NCORES):
        b = c // 4
        h0 = (c % 4) * HPC
        cs = h0 * HD
        w4 = {}
        for nm, W in (("Wq4", Wq), ("Wk4", Wk), ("Wv4", Wv)):
            w4[nm] = swiz(W[h0:h0 + HPC].transpose(1, 0, 2)
                          .reshape(C, HPC * HD).astype(np.float16))
        b6 = np.stack([bq[h0:h0 + HPC].reshape(2, 128),
                       bk[h0:h0 + HPC].reshape(2, 128),
                       bv[h0:h0 + HPC].reshape(2, 128)])  # [3, 2, 128]
        b6 = np.ascontiguousarray(
            b6.reshape(6, 128).T.astype(np.float32))      # [128, 6]
        wpt = np.ascontiguousarray(
            Wp[:, cs:cs + HPC * HD].T.astype(np.float16)
            .reshape(2, 128, C).transpose(1, 0, 2))       # [128, 2, C]
        in_maps.append({
            "xT4": xTs[b], "eT4": eTs[b],
            "Wq4": w4["Wq4"], "Wk4": w4["Wk4"], "Wv4": w4["Wv4"],
            "b6": b6, "WpT4": wpt,
        })
    return in_maps


def kernel(e, x, Wq, bq, Wk, bk, Wv, bv, Wp):
    global LAST_RESULTS
    nc = _get_nc()
    in_maps = make_in_maps(e, x, Wq, bq, Wk, bk, Wv, bv, Wp)
    res = run_bass_kernel_spmd(
        nc, in_maps, list(range(NCORES)),
        trace=bool(os.environ.get("BASS_TRACE")),
    )
    LAST_RESULTS = res
    out = np.zeros((B, T, C), dtype=np.float32)
    for c in range(NCORES):
        out[c // 4] += res.results[c]["y"].astype(np.float32)
    return out


# revision 11
# speedup vs baseline: 1.0419x; 1.0419x over previous
"""Multi-head cross-attention Trainium2 kernel (8-core SPMD), v3.

Sharding: 2 batch groups x 4 cores. Core c handles batch b = c // 4 and
heads [4*(c%4), 4*(c%4)+4). Each core computes its 4 heads' attention
output and a partial output projection (row-sharded Wp); the host sums
the 4 partials per batch (the all-reduce step of tensor parallelism).

All matmul operands are fp16 (fp32 matmul is 4x slower), accumulation is
fp32 in PSUM. Output y is written fp16 (error budget 2e-2 >> fp16 eps).

The kernel is PE-issue-bound (~215ns per 512-col matmul slot, LDWEIGHTS
hidden) with ACT exp (~1.11us per s-tile) a close second. v3 removes all
non-essential PE slots and overlaps the head/tail:
 - per-tq/per-sc chunked input DMAs; Q/KV priming starts ~4us in.
 - PE warm-up matmuls during the DMA window (HAM K=8/8 by first real MM).
 - V^T -> V_aug via PE transpose + DVE copies (DMA-transpose XBAR costs
   ~1.2us of serial DGE-queue time per 64-col transpose -- too slow).
 - softmax denominator broadcast via DRAM-bounce DMA on the gpsimd queue
   (frees 16 PE broadcast matmuls and all tq-boundary PSUM pressure).
 - exp ACT table preloaded at t=0; fp16 output with per-t-tile DMAs.
 - deferred Q/KV/outproj work split into ~1-matmul items popped into the
   exp slack, with deadline-forced emission (+2 iteration margin).

Structure per core ("mh" = head pair, 2 per core):
  Q^T[d,t] = Wq4.T @ xT (+bias)  per (mh, tq) chunk
  K^T/V^T chunks; V_aug[s, 64|1(ones)] via DMA-transpose
  attention (ACT-bound): att[s-tile] = [h0 | h1] joint PSUM [128,1024],
    E = exp(att/32) one ACT instr, fp16; U_aug[65,tq] += V_aug.T @ E
    (softmax denom rides in row 64).
  normalize: U evac, DRAM-bounce denom broadcast, recip, mul.
  outproj: y_ps[n] += UN[mh].T @ WpT; fp16 evac; per-t-tile DMA.

PSUM: "att" 2x2 banks + "uh" 2x1 + "kv" 2x1 = 8 banks.
"""

import os
import numpy as np
from collections import deque
from contextlib import ExitStack

import concourse.bass as bass
import concourse.bacc as bacc
import concourse.tile as tile
from concourse import mybir
from concourse.bass_utils import run_bass_kernel_spmd
from concourse.masks import make_identity

F32 = mybir.dt.float32
F16 = mybir.dt.float16
AF = mybir.ActivationFunctionType

B, T, S, C = 2, 2048, 2048, 1024
H, HD = 16, 64
NCORES = 8
HPC = 4            # heads per core
MHN = 2            # head-pairs per core
KC = C // 128      # 8 contraction tiles
STILES = S // 128  # 16
TTILES = T // 16   # 128
TQN = 4            # t-quarters of 512
SCN = 4            # s-chunks of 512
SCALE = 1.0 / np.sqrt(C)
WARMUP_MMS = 12    # PE warm-up matmuls during the input-DMA window

LAST_RESULTS = None
_NC_CACHE = None


def _build_nc():
    nc = bacc.Bacc()

    xT4 = nc.declare_dram_parameter("xT4", [128, TQN, KC, 512], F16, isOutput=False)
    eT4 = nc.declare_dram_parameter("eT4", [128, SCN, KC, 512], F16, isOutput=False)
    Wq4 = nc.declare_dram_parameter("Wq4", [128, KC, 256], F16, isOutput=False)
    Wk4 = nc.declare_dram_parameter("Wk4", [128, KC, 256], F16, isOutput=False)
    Wv4 = nc.declare_dram_parameter("Wv4", [128, KC, 256], F16, isOutput=False)
    b6 = nc.declare_dram_parameter("b6", [128, 6], F32, isOutput=False)
    WpT4 = nc.declare_dram_parameter("WpT4", [128, 2, C], F16, isOutput=False)
    y = nc.declare_dram_parameter("y", [T, C], F16, isOutput=True)
    # denominator DRAM-bounce scratch, one slot per (mh, tq)
    dscr = nc.declare_dram_parameter("dscr", [MHN * TQN, 1024], F32,
                                     isOutput=True)

    with tile.TileContext(nc) as tc, ExitStack() as ctx:
        consts = ctx.enter_context(tc.tile_pool(name="consts", bufs=1))
        wpool = ctx.enter_context(tc.tile_pool(name="wts", bufs=1))
        qkvp = ctx.enter_context(tc.tile_pool(name="qkvt", bufs=2))
        vtsp = ctx.enter_context(tc.tile_pool(name="vts", bufs=2))
        vap = ctx.enter_context(tc.tile_pool(name="vaug", bufs=2))
        epool = ctx.enter_context(tc.tile_pool(name="esb", bufs=4))
        unp = ctx.enter_context(tc.tile_pool(name="unorm", bufs=2))
        usbp = ctx.enter_context(tc.tile_pool(name="usb", bufs=3))
        dnp = ctx.enter_context(tc.tile_pool(name="denom", bufs=2))
        ysbp = ctx.enter_context(tc.tile_pool(name="ysb", bufs=3))
        psp = ctx.enter_context(tc.tile_pool(name="ps", bufs=2, space="PSUM"))

        # ---- t=0: ACT exp table preload ----
        dummy = consts.tile([1, 8], F32, tag="dummy", name="dummy")
        nc.vector.memset(dummy, 0.0)
        nc.scalar.activation(dummy, dummy, AF.Exp, scale=1.0)

        ident = consts.tile([128, 128], F16)
        make_identity(nc, ident)
        ones1 = consts.tile([128, 64], F32, tag="ones1", name="ones1")
        nc.gpsimd.memset(ones1, 1.0)

        b6sb = consts.tile([128, 6], F32, tag="b6", name="b6sb")
        nc.sync.dma_start(out=b6sb, in_=b6[:, :])
        bsb = {"q": b6sb[:, 0:2], "k": b6sb[:, 2:4], "v": b6sb[:, 4:6]}

        # ---- input DMAs, ordered by first consumer ----
        wsb = {}
        wsb["q"] = wpool.tile([128, KC, 256], F16, tag="wq", name="wqsb")
        nc.sync.dma_start(out=wsb["q"], in_=Wq4[:, :, :])
        # chunk-major: each chunk DMA dst is one contiguous 8KB run per
        # partition (strided dsts cost ~3.7us of serial DGE queue time)
        xt_sb = wpool.tile([128, TQN, KC, 512], F16, tag="xt")
        et_sb = wpool.tile([128, SCN, KC, 512], F16, tag="et")
        nc.sync.dma_start(out=xt_sb[:, 0], in_=xT4[:, 0])
        wsb["k"] = wpool.tile([128, KC, 256], F16, tag="wk", name="wksb")
        nc.sync.dma_start(out=wsb["k"], in_=Wk4[:, :, :])
        nc.sync.dma_start(out=et_sb[:, 0], in_=eT4[:, 0])
        wsb["v"] = wpool.tile([128, KC, 256], F16, tag="wv", name="wvsb")
        nc.sync.dma_start(out=wsb["v"], in_=Wv4[:, :, :])
        for j in range(1, 4):
            nc.sync.dma_start(out=xt_sb[:, j], in_=xT4[:, j])
            nc.sync.dma_start(out=et_sb[:, j], in_=eT4[:, j])
        wpt = wpool.tile([128, 2, C], F16, tag="wpt")
        nc.sync.dma_start(out=wpt, in_=WpT4[:, :, :])

        # ---- PE warm-up: keep HAM at K=8/8 through the DMA window ----
        wz = consts.tile([128, 512], F16, tag="wz", name="wz")
        nc.vector.memset(wz, 0.0)
        warm_ps = psp.tile([128, 512], F32, tag="kv", name="warm_ps")
        for _ in range(WARMUP_MMS):
            nc.tensor.matmul(warm_ps, wz[:, 0:128], wz,
                             start=True, stop=True)

        # ---- persistent SBUF tensors ----
        QT = [qkvp.tile([128, T], F16, tag="qt", name=f"QT{i}")
              for i in range(MHN)]
        KT = [qkvp.tile([128, S], F16, tag="kt", name=f"KT{i}")
              for i in range(MHN)]
        # 256-wide rows: DMA-transpose dests must sit at 128B multiples
        # (the XBAR shifts writes +16B otherwise). h0 V at 0:64 (+ones
        # col 64), h1 V at 128:192 (+ones col 192).
        VA = [vap.tile([128, STILES, 256], F16, tag="va", name=f"va{i}")
              for i in range(MHN)]
        for i in range(MHN):
            nc.gpsimd.memset(VA[i][:, :, 64:65], 1.0)
            nc.gpsimd.memset(VA[i][:, :, 192:193], 1.0)
        UN = [unp.tile([128, T], F16, tag="un", name=f"UN{i}")
              for i in range(MHN)]

        def q_pass(mh, tq):
            """Generator of work items: Q^T for one (mh, tq)."""
            state = {}

            def mk_alloc():
                state["ps"] = psp.tile([128, 512], F32, tag="kv",
                                       name=f"qtps{mh}_{tq}")

            yield mk_alloc
            for k in range(KC):
                def mk_k(k=k):
                    nc.tensor.matmul(state["ps"],
                                     wsb["q"][:, k, mh * 128:(mh + 1) * 128],
                                     xt_sb[:, tq, k],
                                     start=(k == 0), stop=(k == KC - 1))

                yield mk_k

            def mk_evac():
                nc.vector.tensor_scalar_add(
                    out=QT[mh][:, tq * 512:(tq + 1) * 512],
                    in0=state["ps"], scalar1=bsb["q"][:, mh:mh + 1])

            yield mk_evac

        def kv_work(mh, sc):
            """Generator of work items: K^T/V^T chunk + V DMA-transposes."""
            csl = slice(sc * 512, (sc + 1) * 512)
            state = {}

            def mk_alloc():
                state["kt_ps"] = psp.tile([128, 512], F32, tag="kv",
                                          name=f"ktps{mh}_{sc}")
                state["vt_ps"] = psp.tile([128, 512], F32, tag="kv",
                                          name=f"vtps{mh}_{sc}")

            yield mk_alloc

            for nm in ("k", "v"):
                for k in range(KC):
                    def mk_k(nm=nm, k=k):
                        ps = state["kt_ps"] if nm == "k" else state["vt_ps"]
                        nc.tensor.matmul(ps,
                                         wsb[nm][:, k, mh * 128:(mh + 1) * 128],
                                         et_sb[:, sc, k],
                                         start=(k == 0), stop=(k == KC - 1))

                    yield mk_k

            def mk_evac():
                nc.vector.tensor_scalar_add(out=KT[mh][:, csl],
                                            in0=state["kt_ps"],
                                            scalar1=bsb["k"][:, mh:mh + 1])
                vts = vtsp.tile([128, 512], F16, tag="vts",
                                name=f"vts{mh}_{sc}")
                nc.vector.tensor_scalar_add(out=vts, in0=state["vt_ps"],
                                            scalar1=bsb["v"][:, mh:mh + 1])
                state["vts"] = vts

            yield mk_evac

            def mk_tr(jj):
                for j in (jj, jj + 1):
                    s = sc * 4 + j
                    blk = slice(j * 128, (j + 1) * 128)
                    if mh == 1:
                        # long lead time: idle sync-queue DMA-transpose
                        nc.sync.dma_start(out=VA[mh][:, s, 0:64],
                                          in_=state["vts"][0:64, blk],
                                          transpose=True)
                        nc.sync.dma_start(out=VA[mh][:, s, 128:192],
                                          in_=state["vts"][64:128, blk],
                                          transpose=True)
                    else:
                        vtr = psp.tile([128, 128], F16, tag="kv",
                                       name=f"vtr{mh}_{s}")
                        nc.tensor.transpose(
                            vtr, state["vts"][:, blk], ident)
                        nc.vector.tensor_copy(VA[mh][:, s, 0:64],
                                              vtr[:, 0:64])
                        nc.vector.tensor_copy(VA[mh][:, s, 128:192],
                                              vtr[:, 64:128])

            yield (lambda: mk_tr(0))
            yield (lambda: mk_tr(2))

        y_r = y.rearrange("(tt p) o -> tt p o", p=128)

        def outproj_work(tq):
            """Deferred: partial out-projection for one t-quarter,
            per t-tile: 2 matmul items + 1 evac item + 1 dma item."""
            for j in range(4):
                t = tq * 4 + j
                state = {}

                def mk_mm(n, t=t, state=state):
                    if n == 0:
                        state["ps"] = [
                            psp.tile([128, 512], F32, tag="kv",
                                     name=f"yps{t}_{i}") for i in range(2)]
                    for mh in range(MHN):
                        nc.tensor.matmul(
                            state["ps"][n], UN[mh][:, t * 128:(t + 1) * 128],
                            wpt[:, mh, n * 512:(n + 1) * 512],
                            start=(mh == 0), stop=(mh == MHN - 1))

                def mk_evac(t=t, state=state):
                    ysb = ysbp.tile([128, 1024], F16, tag="ysb",
                                    name=f"ysb{t}")
                    nc.vector.tensor_copy(ysb[:, 0:512], state["ps"][0])
                    nc.vector.tensor_copy(ysb[:, 512:1024], state["ps"][1])
                    state["ysb"] = ysb

                def mk_dma(t=t, state=state):
                    nc.sync.dma_start(out=y_r[t], in_=state["ysb"])

                yield (True, lambda f=mk_mm: f(0))
                yield (True, lambda f=mk_mm: f(1))
                yield (False, mk_evac)
                yield (False, mk_dma)

        def normalize_work(mh, tq, uh0, uh1, last=False):
            """Deferred: evac U, denom broadcast (DRAM-bounce; PE matmul
            when `last` -- PSUM is free and latency matters), recip, mul."""
            qsl = slice(tq * 512, (tq + 1) * 512)
            slot = mh * TQN + tq
            state = {}

            def mk_evac0():
                usb = usbp.tile([65, 1024], F32, tag="usb",
                                name=f"usb{mh}_{tq}")
                nc.vector.tensor_copy(usb[:, 0:512], uh0)
                state["usb"] = usb

            def mk_evac1():
                nc.vector.tensor_copy(state["usb"][:, 512:1024], uh1)

            def mk_bounce():
                if last:
                    bcps = psp.tile([64, 1024], F32, tag="att",
                                    name=f"bcps{mh}_{tq}")
                    usb = state["usb"]
                    nc.tensor.matmul(bcps[:, 0:512], ones1[64:65, :],
                                     usb[64:65, 0:512], start=True, stop=True)
                    nc.tensor.matmul(bcps[:, 512:1024], ones1[64:65, :],
                                     usb[64:65, 512:1024], start=True,
                                     stop=True)
                    state["bc"] = bcps
                    return
                # denom row 64 -> DRAM -> broadcast-read to 64 partitions
                nc.gpsimd.dma_start(out=dscr[slot:slot + 1, :],
                                    in_=state["usb"][64:65, :])
                bc = dnp.tile([64, 1024], F32, tag="bc", bufs=2,
                              name=f"bc{mh}_{tq}")
                nc.gpsimd.dma_start(
                    out=bc, in_=dscr[slot:slot + 1, :].to_broadcast([64, 1024]))
                state["bc"] = bc

            def mk_recip():
                rbc = dnp.tile([64, 1024], F32, tag="rbc", bufs=2,
                               name=f"rbc{mh}_{tq}")
                nc.vector.reciprocal_approx_fast(rbc, state["bc"])
                state["rbc"] = rbc

            def mk_div0():
                nc.vector.tensor_mul(UN[mh][0:64, qsl],
                                     state["usb"][0:64, 0:512],
                                     state["rbc"][:, 0:512])

            def mk_div1():
                tmp1 = dnp.tile([64, 512], F16, tag="tmp1", bufs=2,
                                name=f"tmp1_{mh}_{tq}")
                nc.vector.tensor_mul(tmp1, state["usb"][0:64, 512:1024],
                                     state["rbc"][:, 512:1024])
                nc.gpsimd.dma_start(out=UN[mh][64:128, qsl], in_=tmp1)

            yield (False, mk_evac0)
            yield (False, mk_evac1)
            yield (last, mk_bounce)
            yield (False, mk_recip)
            yield (False, mk_div0)
            yield (False, mk_div1)

        emitted = set()
        bulk_groups = {}
        bulk_order = []

        def pop1():
            key = bulk_order[0]
            g = bulk_groups[key]
            g.popleft()()
            if not g:
                bulk_order.pop(0)
                emitted.add(key)

        def need(key):
            """Force-emit all remaining items of producer group `key`."""
            if key in emitted or key not in bulk_groups:
                return
            g = bulk_groups[key]
            while g:
                g.popleft()()
            bulk_order.remove(key)
            emitted.add(key)

        def attention(mh, fast, _unused, bulk_budget):
            """ACT-bound attention for head pair mh; pops deferred work
            into the PE slack: 1 fast item + up to bulk_budget bulk."""
            for tq in range(TQN):
                need(("q", mh, tq))
                qsl = slice(tq * 512, (tq + 1) * 512)
                uh0 = psp.tile([65, 512], F32, tag="uh", name=f"uh0_{mh}_{tq}")
                uh1 = psp.tile([65, 512], F32, tag="uh", name=f"uh1_{mh}_{tq}")
                prev_av = None
                for s in range(STILES):
                    # +2 iteration margin for the VA transpose latency
                    need(("kv", mh, min((s + 2) // 4, SCN - 1)))
                    att = psp.tile([128, 1024], F32, tag="att",
                                   name=f"att_{mh}_{tq}_{s}")
                    ssl = slice(s * 128, (s + 1) * 128)
                    nc.tensor.matmul(att[:, 0:512], KT[mh][0:64, ssl],
                                     QT[mh][0:64, qsl], start=True, stop=True)
                    nc.tensor.matmul(att[:, 512:1024], KT[mh][64:128, ssl],
                                     QT[mh][64:128, qsl], start=True, stop=True)
                    ej = epool.tile([128, 1024], F16, tag="e",
                                    name=f"e_{mh}_{tq}_{s}")
                    nc.scalar.activation(ej, att, AF.Exp, scale=float(SCALE))
                    if prev_av is not None:
                        prev_av()
                    popped = pe_popped = 0
                    for _ in range(len(fast)):
                        if popped >= 2:
                            break
                        is_pe, fn = fast[0]
                        if is_pe and pe_popped >= 1:
                            break
                        fast.popleft()
                        fn()
                        popped += 1
                        pe_popped += is_pe
                    while bulk_order and popped < bulk_budget:
                        pop1()
                        popped += 1

                    def mk_av(s=s, ej=ej, uh0=uh0, uh1=uh1):
                        nc.tensor.matmul(uh0, VA[mh][:, s, 0:65], ej[:, 0:512],
                                         start=(s == 0), stop=(s == STILES - 1))
                        nc.tensor.matmul(uh1, VA[mh][:, s, 128:193],
                                         ej[:, 512:1024],
                                         start=(s == 0), stop=(s == STILES - 1))
                    prev_av = mk_av
                prev_av()
                fast.extend(normalize_work(mh, tq, uh0, uh1,
                                           last=(mh == MHN - 1
                                                 and tq == TQN - 1)))
                if mh == MHN - 1:
                    fast.extend(outproj_work(tq))
            return fast

        # ---- priming: first chunks of both pairs while DMAs stream ----
        for w in q_pass(0, 0):
            w()
        for w in q_pass(1, 0):
            w()
        for w in kv_work(0, 0):
            w()
        emitted.update({("q", 0, 0), ("q", 1, 0), ("kv", 0, 0)})

        # ---- deferred bulk: priority order; need() pulls groups OOO ----
        for mh, kind, idx in (
            (1, "kv", 0),
            (0, "kv", 1),
            (1, "kv", 1),
            (0, "kv", 2),
            (1, "kv", 2),
            (0, "kv", 3),
            (1, "kv", 3),
            (0, "q", 1), (0, "q", 2), (0, "q", 3),
            (1, "q", 1), (1, "q", 2), (1, "q", 3),
        ):
            key = (kind, mh, idx)
            gen = q_pass(mh, idx) if kind == "q" else kv_work(mh, idx)
            bulk_groups[key] = deque(gen)
            bulk_order.append(key)

        fast = deque()
        fast = attention(0, fast, None, bulk_budget=2)
        fast = attention(1, fast, None, bulk_budget=2)
        for _ in range(len(fast)):
            is_pe, fn = fast.popleft()
            fn()
        while bulk_order:
            pop1()

    nc.compile()
    return nc


def _get_nc():
    global _NC_CACHE
    if _NC_CACHE is None:
        _NC_CACHE = _build_nc()
    return _NC_CACHE


def make_in_maps(e, x, Wq, bq, Wk, bk, Wv, bv, Wp):
    e = np.asarray(e, dtype=np.float32)
    x = np.asarray(x, dtype=np.float32)
    Wq, bq = np.asarray(Wq, np.float32), np.asarray(bq, np.float32)
    Wk, bk = np.asarray(Wk, np.float32), np.asarray(bk, np.float32)
    Wv, bv = np.asarray(Wv, np.float32), np.asarray(bv, np.float32)
    Wp = np.asarray(Wp, np.float32)

    def swiz(a2d):  # [C, N] -> [128, KC, N] partition-major
        Cd, N = a2d.shape
        return np.ascontiguousarray(
            a2d.reshape(KC, 128, N).transpose(1, 0, 2))

    def swiz4(a2d):  # [C, N] -> [128, 4, KC, 512] chunked partition-major
        return np.ascontiguousarray(
            a2d.reshape(KC, 128, 4, 512).transpose(1, 2, 0, 3))

    xTs = [swiz4(x[b].T.astype(np.float16)) for b in range(B)]
    eTs = [swiz4(e[b].T.astype(np.float16)) for b in range(B)]
    in_maps = []
    for c in range(NCORES):
        b = c // 4
        h0 = (c % 4) * HPC
        cs = h0 * HD
        w4 = {}
        for nm, W in (("Wq4", Wq), ("Wk4", Wk), ("Wv4", Wv)):
            w4[nm] = swiz(W[h0:h0 + HPC].transpose(1, 0, 2)
                          .reshape(C, HPC * HD).astype(np.float16))
        b6 = np.stack([bq[h0:h0 + HPC].reshape(2, 128),
                       bk[h0:h0 + HPC].reshape(2, 128),
                       bv[h0:h0 + HPC].reshape(2, 128)])  # [3, 2, 128]
        b6 = np.ascontiguousarray(
            b6.reshape(6, 128).T.astype(np.float32))      # [128, 6]
        wpt = np.ascontiguousarray(
            Wp[:, cs:cs + HPC * HD].T.astype(np.float16)
            .reshape(2, 128, C).transpose(1, 0, 2))       # [128, 2, C]
        in_maps.append({
            "xT4": xTs[b], "eT4": eTs[b],
            "Wq4": w4["Wq4"], "Wk4": w4["Wk4"], "Wv4": w4["Wv4"],
            "b6": b6, "WpT4": wpt,
        })
    return in_maps


def kernel(e, x, Wq, bq, Wk, bk, Wv, bv, Wp):
    global LAST_RESULTS
    nc = _get_nc()
    in_maps = make_in_maps(e, x, Wq, bq, Wk, bk, Wv, bv, Wp)
    res = run_bass_kernel_spmd(
        nc, in_maps, list(range(NCORES)),
        trace=bool(os.environ.get("BASS_TRACE")),
    )
    LAST_RESULTS = res
    out = np.zeros((B, T, C), dtype=np.float32)
    for c in range(NCORES):
        out[c // 4] += res.results[c]["y"].astype(np.float32)
    return out
